# revision 10
# baseline (speedup 1.0000x reference)
"""Multi-head causal attention (B=4, T=2048, H=16, D=64) on 8 trn2 NeuronCores.

Sharding: core c = (batch b = c//2, head-group hg = c%2 of 8 heads).
Each core computes its batch's QKV projection for its 8 heads, causal
attention, and a partial output projection (contraction over its 512
channels of W_proj). Host sums the two partials per batch and adds bias.

Per-core kernel layout choices (see comments inline):
  - x is passed pre-transposed as xT [C=1024, T=2048].
  - K^T, Q^T stored [hd, t] with head-dim on partitions (64 per head, 2
    heads per 128-partition tile) -> scores matmuls row-pack 2 heads.
  - V stored [t, h*65+d] with a ones column appended per head -> the AV
    matmul O_T = V_aug^T(stationary) x P_T produces softmax denominators
    in row 64 for free.
  - Scores are computed transposed S_T[k, q] so that P_T = exp(S_T) is
    directly the AV matmul's moving operand; softmax needs no max
    subtraction (|scores/8| < ~3) and no separate sum reduction.
  - O_T [hd, t] is exactly the lhsT the output projection needs.
"""

import os
import sys

import numpy as np

BF16_NP = np.dtype(np.float16)

if "/opt/trn_rl_repo" not in sys.path:
    sys.path.insert(0, "/opt/trn_rl_repo")

from contextlib import ExitStack

import concourse.bass as bass
import concourse.bacc as bacc
import concourse.mybir as mybir
import concourse.tile as tile
from concourse._compat import with_exitstack

P = 128
T = 2048
C = 1024
H_PER_CORE = 8
D = 64
DP = D + 1  # V augmented with a ones column
NC_CORES = 8

TB = 4  # t-blocks of 512
QB = 4  # q-blocks of 512
NKT = 16  # k-tiles of 128
CI = 8  # contraction tiles of 128 over C for QKV proj

F32 = mybir.dt.float32
BF16 = mybir.dt.float16  # fp16: full matmul rate, 8x finer mantissa than bf16


def _mm(ap):
    return ap


@with_exitstack
def build_attention_kernel(ctx: ExitStack, tc: tile.TileContext):
    nc = tc.nc

    xT = nc.declare_dram_parameter("xT", [C, T], BF16, isOutput=False)
    wk = nc.declare_dram_parameter("wk", [C, 512], BF16, isOutput=False)
    wq = nc.declare_dram_parameter("wq", [C, 512], BF16, isOutput=False)
    wv = nc.declare_dram_parameter("wv", [C, 512], BF16, isOutput=False)
    wp = nc.declare_dram_parameter("wp", [512, C], BF16, isOutput=False)
    y = nc.declare_dram_parameter("y", [T, C], F32, isOutput=True)

    xT_t = xT.rearrange("(co ci) t -> ci co t", ci=P)
    wk_t = wk.rearrange("(co ci) m -> ci co m", ci=P)
    wq_t = wq.rearrange("(co ci) m -> ci co m", ci=P)
    wv_t = wv.rearrange("(co ci) m -> ci co m", ci=P)
    wp_t = wp.rearrange("(co ci) n -> ci co n", ci=P)

    # ---- persistent SBUF pools ----
    # Fine-grained tiles so phase-2 consumers only wait on the exact
    # phase-1 producers they need.
    kt_pool = ctx.enter_context(tc.tile_pool(name="ktp", bufs=16))
    qt_pool = ctx.enter_context(tc.tile_pool(name="qtp", bufs=16))
    v_pool = ctx.enter_context(tc.tile_pool(name="vp", bufs=4))
    const_pool = ctx.enter_context(tc.tile_pool(name="constp", bufs=1))

    # KT[pt][tb], QT[pt][qb]: [128, 512]; partitions = 2 heads x 64 dims
    KT = [[kt_pool.tile([P, 512], BF16, tag="kt", name=f"KT_{pt}_{tb}") for tb in range(TB)] for pt in range(4)]
    QT = [[qt_pool.tile([P, 512], BF16, tag="qt", name=f"QT_{pt}_{qb}") for qb in range(QB)] for pt in range(4)]
    # V[tb]: [128, 4(t-subtile), 8*65]
    V = [v_pool.tile([P, 4, H_PER_CORE * DP], BF16, tag="v", name=f"V_{tb}") for tb in range(TB)]

    masks = const_pool.tile([P, 4, 512], BF16, tag="masks", name="masks")

    # diagonal causal masks: masks[:, j, :][kk, qq] = 1.0 if qq >= kk + j*128
    for j in range(4):
        nc.gpsimd.memset(masks[:, j, :], 1.0)
        nc.gpsimd.affine_select(
            out=masks[:, j, :],
            in_=masks[:, j, :],
            compare_op=mybir.AluOpType.is_ge,
            fill=0.0,
            base=-j * P,
            pattern=[[1, 512]],  # +1 per q element
            channel_multiplier=-1,  # -1 per k partition
        )
    # ones column of V
    for tb in range(TB):
        ones_col = V[tb].rearrange("p s (h e) -> p s h e", e=DP)[:, :, :, D : D + 1]
        nc.gpsimd.memset(ones_col, 1.0)

    # =========================== phase 1: QKV ===========================
    with (
        tc.tile_pool(name="wkp", bufs=1) as wk_pool,
        tc.tile_pool(name="wqp", bufs=1) as wq_pool,
        tc.tile_pool(name="wvp", bufs=1) as wv_pool,
        tc.tile_pool(name="xtp", bufs=2) as xt_pool,
        tc.tile_pool(name="ps1", bufs=4, space="PSUM") as ps1_pool,
    ):
        wk_sb = wk_pool.tile([P, CI, 512], BF16)
        wq_sb = wq_pool.tile([P, CI, 512], BF16)
        wv_sb = wv_pool.tile([P, CI, 512], BF16)
        nc.sync.dma_start(wk_sb[:], wk_t)
        xts = []
        for tb in range(2):  # prefetch first two t-blocks before wq/wv
            xt = xt_pool.tile([P, CI, 512], BF16, tag="xt", name="xt")
            nc.sync.dma_start(xt[:], xT_t[:, :, tb * 512 : (tb + 1) * 512])
            xts.append(xt)
        nc.sync.dma_start(wq_sb[:], wq_t)
        nc.sync.dma_start(wv_sb[:], wv_t)

        for tb in range(TB):
            if tb < 2:
                xt = xts[tb]
            else:
                xt = xt_pool.tile([P, CI, 512], BF16, tag="xt", name="xt")
                nc.sync.dma_start(xt[:], xT_t[:, :, tb * 512 : (tb + 1) * 512])

            for pt in range(4):  # K^T: lhsT = wk cols, rhs = xT
                ps = ps1_pool.tile([P, 512], F32, tag="ps1", name="ps")
                for ci in range(CI):
                    nc.tensor.matmul(
                        ps[:],
                        lhsT=_mm(wk_sb[:, ci, pt * P : (pt + 1) * P]),
                        rhs=_mm(xt[:, ci, :]),
                        start=(ci == 0),
                        stop=(ci == CI - 1),
                    )
                nc.scalar.copy(KT[pt][tb][:], ps[:])
            for pt in range(4):  # Q^T
                ps = ps1_pool.tile([P, 512], F32, tag="ps1", name="ps")
                for ci in range(CI):
                    nc.tensor.matmul(
                        ps[:],
                        lhsT=_mm(wq_sb[:, ci, pt * P : (pt + 1) * P]),
                        rhs=_mm(xt[:, ci, :]),
                        start=(ci == 0),
                        stop=(ci == CI - 1),
                    )
                nc.vector.tensor_copy(QT[pt][tb][:], ps[:])
            for ts_ in range(4):  # V: lhsT = xT t-slice, rhs = wv
                ps = ps1_pool.tile([P, 512], F32, tag="ps1", name="ps")
                for ci in range(CI):
                    nc.tensor.matmul(
                        ps[:],
                        lhsT=_mm(xt[:, ci, ts_ * P : (ts_ + 1) * P]),
                        rhs=_mm(wv_sb[:, ci, :]),
                        start=(ci == 0),
                        stop=(ci == CI - 1),
                    )
                nc.vector.tensor_copy(
                    V[tb][:, ts_].rearrange("p (h e) -> p h e", e=DP)[:, :, :D],
                    ps.rearrange("p (h d) -> p h d", d=D),
                )

    # ========================= phase 2: attention =========================
    # OT pool opened after phase-1 pools closed: reuses their SBUF space.
    ot_pool = ctx.enter_context(tc.tile_pool(name="otp", bufs=16))
    # OT[hp][qb]: [128, 512] attention out (normalized), same layout as KT
    OT = [[ot_pool.tile([P, 512], BF16, tag="ot", name=f"OT_{hp}_{qb}") for qb in range(QB)] for hp in range(4)]
    with (
        tc.tile_pool(name="ptp", bufs=4) as pt_pool,
        tc.tile_pool(name="recipp", bufs=4) as recip_pool,
        tc.tile_pool(name="bcp", bufs=2) as bc_pool,
        tc.tile_pool(name="ps_s", bufs=2, space="PSUM") as ps_s_pool,
        tc.tile_pool(name="ps_o", bufs=3, space="PSUM") as ps_o_pool,
    ):
        for qb in range(QB):
            for hp in range(4):
                ot_ps = [ps_o_pool.tile([DP, 512], F32, tag="ot_ps", name=f"ot_ps_{i}") for i in range(2)]
                nkt = 4 * (qb + 1)
                for kt in range(nkt):
                    tb = kt // 4
                    # both heads' S_T side by side in one 2-bank psum tile
                    s_ps = ps_s_pool.tile([P, 2, 512], F32, tag="s_ps", name="s_ps")
                    for h2 in range(2):
                        # S_T[k, q] for head h = 2*hp + h2 (row-packed pair)
                        nc.tensor.matmul(
                            s_ps[:, h2, :],
                            lhsT=_mm(
                                KT[hp][tb][
                                    h2 * D : (h2 + 1) * D,
                                    (kt % 4) * P : (kt % 4 + 1) * P,
                                ]
                            ),
                            rhs=_mm(QT[hp][qb][h2 * D : (h2 + 1) * D, :]),
                            start=True,
                            stop=True,
                        )
                    # one exp over both heads (2 banks, 1024 elems/partition)
                    p_t = pt_pool.tile([P, 2, 512], BF16, tag="pt", name="p_t")
                    nc.scalar.activation(
                        p_t[:],
                        s_ps[:],
                        mybir.ActivationFunctionType.Exp,
                        scale=0.125,
                    )
                    if kt >= 4 * qb:  # diagonal tile: zero q < k entries
                        j = kt - 4 * qb
                        mb = masks[:, j : j + 1, :].to_broadcast([P, 2, 512])
                        nc.vector.tensor_mul(p_t[:], p_t[:], mb)
                    for h2 in range(2):
                        h = 2 * hp + h2
                        nc.tensor.matmul(
                            ot_ps[h2][:],
                            lhsT=_mm(
                                V[tb][:, kt % 4, h * DP : (h + 1) * DP]
                            ),
                            rhs=_mm(p_t[:, h2, :]),
                            start=(kt == 0),
                            stop=(kt == nkt - 1),
                        )
                # normalize: divide rows 0..63 by the sums row (64)
                for h2 in range(2):
                    recip = recip_pool.tile([1, 512], F32, tag="recip", name="recip")
                    nc.vector.reciprocal(recip[:], ot_ps[h2][D : D + 1, :])
                    bc = bc_pool.tile([D, 512], F32, tag="bc", name="bc")
                    nc.gpsimd.partition_broadcast(bc[:], recip[:])
                    nc.vector.tensor_mul(
                        OT[hp][qb][h2 * D : (h2 + 1) * D, :],
                        ot_ps[h2][:D, :],
                        bc[:],
                    )

    # ========================= phase 3: out proj =========================
    with (
        tc.tile_pool(name="wpp", bufs=1) as wp_pool,
        tc.tile_pool(name="yp", bufs=2) as y_pool,
        tc.tile_pool(name="ps_y", bufs=2, space="PSUM") as ps_y_pool,
    ):
        wp_sb = wp_pool.tile([P, 4, C], BF16)
        nc.sync.dma_start(wp_sb[:], wp_t)
        y_t = y.rearrange("(tt p) n -> p tt n", p=P)
        ysbs = []
        for tt in range(16):
            qb, sub = tt // 4, tt % 4
            y_ps = [ps_y_pool.tile([P, 512], F32, tag="y_ps", name=f"y_ps_{i}") for i in range(2)]
            for ct in range(4):
                lhsT = OT[ct][qb][:, sub * P : (sub + 1) * P]
                for nb in range(2):
                    nc.tensor.matmul(
                        y_ps[nb][:],
                        lhsT=_mm(lhsT),
                        rhs=_mm(wp_sb[:, ct, nb * 512 : (nb + 1) * 512]),
                        start=(ct == 0),
                        stop=(ct == 3),
                    )
            if tt % 2 == 0:
                ypair = y_pool.tile([P, 2, C], F32, tag="ypair", name="ypair")
                ysbs.append(ypair)
            for nb in range(2):
                nc.vector.tensor_copy(
                    ysbs[-1][:, tt % 2, nb * 512 : (nb + 1) * 512], y_ps[nb][:]
                )
            if tt % 2 == 1:  # one DMA per two t-tiles
                nc.sync.dma_start(y_t[:, tt - 1 : tt + 1, :], ysbs[-1][:])

    return nc


_CACHED_NC = None


def get_nc():
    global _CACHED_NC
    if _CACHED_NC is None:
        nc = bacc.Bacc()
        with tile.TileContext(nc) as tc:
            build_attention_kernel(tc)
        nc.compile()
        _CACHED_NC = nc
    return _CACHED_NC


def make_in_maps(x, W_att, W_proj):
    x = np.asarray(x, dtype=np.float32)
    W_att = np.asarray(W_att, dtype=np.float32)
    in_maps = []
    for c in range(NC_CORES):
        b, hg = c // 2, c % 2
        s = hg * 512
        in_maps.append(
            {
                "xT": np.ascontiguousarray(x[b].T).astype(BF16_NP),
                "wk": np.ascontiguousarray(
                    W_att[:, 0 * C + s : 0 * C + s + 512]
                ).astype(BF16_NP),
                "wq": np.ascontiguousarray(
                    W_att[:, 1 * C + s : 1 * C + s + 512]
                ).astype(BF16_NP),
                "wv": np.ascontiguousarray(
                    W_att[:, 2 * C + s : 2 * C + s + 512]
                ).astype(BF16_NP),
                "wp": np.ascontiguousarray(
                    np.asarray(W_proj, np.float32)[s : s + 512]
                ).astype(BF16_NP),
            }
        )
    return in_maps


def combine_outputs(results, b_proj):
    B = NC_CORES // 2
    out = np.empty((B, T, C), dtype=np.float32)
    bias = np.asarray(b_proj, dtype=np.float32)
    for b in range(B):
        out[b] = results[2 * b]["y"] + results[2 * b + 1]["y"] + bias
    return out


def kernel(x, W_att, W_proj, b_proj):
    from concourse.bass_utils import run_bass_kernel_spmd

    nc = get_nc()
    in_maps = make_in_maps(x, W_att, W_proj)
    res = run_bass_kernel_spmd(nc, in_maps, list(range(NC_CORES)))
    return combine_outputs(res.results, b_proj)


# revision 11
# speedup vs baseline: 1.0096x; 1.0096x over previous
"""Multi-head causal attention (B=4, T=2048, H=16, D=64) on 8 trn2 NeuronCores.

Sharding: core c = (batch b = c//2, head-group hg = c%2 of 8 heads).
Each core computes its batch's QKV projection for its 8 heads, causal
attention, and a partial output projection (contraction over its 512
channels of W_proj). Host sums the two partials per batch and adds bias.

Per-core kernel layout choices (see comments inline):
  - x is passed pre-transposed as xT [C=1024, T=2048].
  - K^T, Q^T stored [hd, t] with head-dim on partitions (64 per head, 2
    heads per 128-partition tile) -> scores matmuls row-pack 2 heads.
  - V stored [t, h*65+d] with a ones column appended per head -> the AV
    matmul O_T = V_aug^T(stationary) x P_T produces softmax denominators
    in row 64 for free.
  - Scores are computed transposed S_T[k, q] so that P_T = exp(S_T) is
    directly the AV matmul's moving operand; softmax needs no max
    subtraction (|scores/8| < ~3) and no separate sum reduction.
  - O_T [hd, t] is exactly the lhsT the output projection needs.
"""

import os
import sys

import numpy as np

BF16_NP = np.dtype(np.float16)

if "/opt/trn_rl_repo" not in sys.path:
    sys.path.insert(0, "/opt/trn_rl_repo")

from contextlib import ExitStack

import concourse.bass as bass
import concourse.bacc as bacc
import concourse.mybir as mybir
import concourse.tile as tile
from concourse._compat import with_exitstack

P = 128
T = 2048
C = 1024
H_PER_CORE = 8
D = 64
DP = D + 1  # V augmented with a ones column
NC_CORES = 8

TB = 4  # t-blocks of 512
QB = 4  # q-blocks of 512
NKT = 16  # k-tiles of 128
CI = 8  # contraction tiles of 128 over C for QKV proj

F32 = mybir.dt.float32
BF16 = mybir.dt.float16  # fp16: full matmul rate, 8x finer mantissa than bf16


def _mm(ap):
    return ap


@with_exitstack
def build_attention_kernel(ctx: ExitStack, tc: tile.TileContext):
    nc = tc.nc

    xT = nc.declare_dram_parameter("xT", [C, T], BF16, isOutput=False)
    wk = nc.declare_dram_parameter("wk", [C, 512], BF16, isOutput=False)
    wq = nc.declare_dram_parameter("wq", [C, 512], BF16, isOutput=False)
    wv = nc.declare_dram_parameter("wv", [C, 512], BF16, isOutput=False)
    wp = nc.declare_dram_parameter("wp", [512, C], BF16, isOutput=False)
    y = nc.declare_dram_parameter("y", [T, C], F32, isOutput=True)

    xT_t = xT.rearrange("(co ci) t -> ci co t", ci=P)
    wk_t = wk.rearrange("(co ci) m -> ci co m", ci=P)
    wq_t = wq.rearrange("(co ci) m -> ci co m", ci=P)
    wv_t = wv.rearrange("(co ci) m -> ci co m", ci=P)
    wp_t = wp.rearrange("(co ci) n -> ci co n", ci=P)

    # ---- persistent SBUF pools ----
    # Fine-grained tiles so phase-2 consumers only wait on the exact
    # phase-1 producers they need.
    kt_pool = ctx.enter_context(tc.tile_pool(name="ktp", bufs=16))
    qt_pool = ctx.enter_context(tc.tile_pool(name="qtp", bufs=16))
    v_pool = ctx.enter_context(tc.tile_pool(name="vp", bufs=4))
    const_pool = ctx.enter_context(tc.tile_pool(name="constp", bufs=1))

    # KT[pt][tb], QT[pt][qb]: [128, 512]; partitions = 2 heads x 64 dims
    KT = [[kt_pool.tile([P, 512], BF16, tag="kt", name=f"KT_{pt}_{tb}") for tb in range(TB)] for pt in range(4)]
    QT = [[qt_pool.tile([P, 512], BF16, tag="qt", name=f"QT_{pt}_{qb}") for qb in range(QB)] for pt in range(4)]
    # V[tb]: [128, 4(t-subtile), 8*65]
    V = [v_pool.tile([P, 4, H_PER_CORE * DP], BF16, tag="v", name=f"V_{tb}") for tb in range(TB)]

    masks = const_pool.tile([P, 4, 512], BF16, tag="masks", name="masks")

    # diagonal causal masks: masks[:, j, :][kk, qq] = 1.0 if qq >= kk + j*128
    for j in range(4):
        nc.gpsimd.memset(masks[:, j, :], 1.0)
        nc.gpsimd.affine_select(
            out=masks[:, j, :],
            in_=masks[:, j, :],
            compare_op=mybir.AluOpType.is_ge,
            fill=0.0,
            base=-j * P,
            pattern=[[1, 512]],  # +1 per q element
            channel_multiplier=-1,  # -1 per k partition
        )
    # ones column of V
    for tb in range(TB):
        ones_col = V[tb].rearrange("p s (h e) -> p s h e", e=DP)[:, :, :, D : D + 1]
        nc.gpsimd.memset(ones_col, 1.0)

    # =========================== phase 1: QKV ===========================
    with (
        tc.tile_pool(name="wkp", bufs=1) as wk_pool,
        tc.tile_pool(name="wqp", bufs=1) as wq_pool,
        tc.tile_pool(name="wvp", bufs=1) as wv_pool,
        tc.tile_pool(name="xtp", bufs=2) as xt_pool,
        tc.tile_pool(name="ps1", bufs=4, space="PSUM") as ps1_pool,
    ):
        wk_sb = wk_pool.tile([P, CI, 512], BF16)
        wq_sb = wq_pool.tile([P, CI, 512], BF16)
        wv_sb = wv_pool.tile([P, CI, 512], BF16)
        nc.sync.dma_start(wk_sb[:], wk_t)
        xts = []
        for tb in range(2):  # prefetch first two t-blocks before wq/wv
            xt = xt_pool.tile([P, CI, 512], BF16, tag="xt", name="xt")
            nc.sync.dma_start(xt[:], xT_t[:, :, tb * 512 : (tb + 1) * 512])
            xts.append(xt)
        nc.sync.dma_start(wq_sb[:], wq_t)
        nc.sync.dma_start(wv_sb[:], wv_t)

        for tb in range(TB):
            if tb < 2:
                xt = xts[tb]
            else:
                xt = xt_pool.tile([P, CI, 512], BF16, tag="xt", name="xt")
                nc.sync.dma_start(xt[:], xT_t[:, :, tb * 512 : (tb + 1) * 512])

            for pt in range(4):  # K^T: lhsT = wk cols, rhs = xT
                ps = ps1_pool.tile([P, 512], F32, tag="ps1", name="ps")
                for ci in range(CI):
                    nc.tensor.matmul(
                        ps[:],
                        lhsT=_mm(wk_sb[:, ci, pt * P : (pt + 1) * P]),
                        rhs=_mm(xt[:, ci, :]),
                        start=(ci == 0),
                        stop=(ci == CI - 1),
                    )
                nc.scalar.copy(KT[pt][tb][:], ps[:])
            for pt in range(4):  # Q^T
                ps = ps1_pool.tile([P, 512], F32, tag="ps1", name="ps")
                for ci in range(CI):
                    nc.tensor.matmul(
                        ps[:],
                        lhsT=_mm(wq_sb[:, ci, pt * P : (pt + 1) * P]),
                        rhs=_mm(xt[:, ci, :]),
                        start=(ci == 0),
                        stop=(ci == CI - 1),
                    )
                nc.vector.tensor_copy(QT[pt][tb][:], ps[:])
            for ts_ in range(4):  # V: lhsT = xT t-slice, rhs = wv
                ps = ps1_pool.tile([P, 512], F32, tag="ps1", name="ps")
                for ci in range(CI):
                    nc.tensor.matmul(
                        ps[:],
                        lhsT=_mm(xt[:, ci, ts_ * P : (ts_ + 1) * P]),
                        rhs=_mm(wv_sb[:, ci, :]),
                        start=(ci == 0),
                        stop=(ci == CI - 1),
                    )
                nc.vector.tensor_copy(
                    V[tb][:, ts_].rearrange("p (h e) -> p h e", e=DP)[:, :, :D],
                    ps.rearrange("p (h d) -> p h d", d=D),
                )

    # ================== phase 2+3: attention + out-proj ==================
    # OT pool opened after phase-1 pools closed: reuses their SBUF space.
    ot_pool = ctx.enter_context(tc.tile_pool(name="otp", bufs=16))
    # OT[hp][qb]: [128, 512] attention out (normalized), same layout as KT
    OT = [[ot_pool.tile([P, 512], BF16, tag="ot", name=f"OT_{hp}_{qb}") for qb in range(QB)] for hp in range(4)]
    with (
        tc.tile_pool(name="ptp", bufs=4) as pt_pool,
        tc.tile_pool(name="recipp", bufs=4) as recip_pool,
        tc.tile_pool(name="bcp", bufs=2) as bc_pool,
        tc.tile_pool(name="wpp", bufs=1) as wp_pool,
        tc.tile_pool(name="yp", bufs=2) as y_pool,
        tc.tile_pool(name="ps_s", bufs=2, space="PSUM") as ps_s_pool,
        tc.tile_pool(name="ps_o", bufs=2, space="PSUM") as ps_o_pool,
        tc.tile_pool(name="ps_y", bufs=2, space="PSUM") as ps_y_pool,
    ):
        wp_sb = wp_pool.tile([P, 4, C], BF16)
        nc.sync.dma_start(wp_sb[:], wp_t)
        y_t = y.rearrange("(tt p) n -> p tt n", p=P)
        ysbs = []

        def attention_pair(qb, hp):
            ot_ps = [ps_o_pool.tile([DP, 512], F32, tag="ot_ps", name=f"ot_ps_{i}") for i in range(2)]
            nkt = 4 * (qb + 1)
            pts = {}

            def emit_scores_exp(kt):
                tb = kt // 4
                # both heads' S_T side by side in one 2-bank psum tile
                s_ps = ps_s_pool.tile([P, 2, 512], F32, tag="s_ps", name="s_ps")
                for h2 in range(2):
                    # S_T[k, q] for head h = 2*hp + h2 (row-packed pair)
                    nc.tensor.matmul(
                        s_ps[:, h2, :],
                        lhsT=KT[hp][tb][
                            h2 * D : (h2 + 1) * D,
                            (kt % 4) * P : (kt % 4 + 1) * P,
                        ],
                        rhs=QT[hp][qb][h2 * D : (h2 + 1) * D, :],
                        start=True,
                        stop=True,
                    )
                # one exp over both heads (2 banks, 1024 elems/partition)
                p_t = pt_pool.tile([P, 2, 512], BF16, tag="pt", name="p_t")
                nc.scalar.activation(
                    p_t[:],
                    s_ps[:],
                    mybir.ActivationFunctionType.Exp,
                    scale=0.125,
                )
                if kt >= 4 * qb:  # diagonal tile: zero q < k entries
                    j = kt - 4 * qb
                    mb = masks[:, j : j + 1, :].to_broadcast([P, 2, 512])
                    nc.vector.tensor_mul(p_t[:], p_t[:], mb)
                pts[kt] = p_t

            def emit_av(kt):
                tb = kt // 4
                p_t = pts.pop(kt)
                for h2 in range(2):
                    h = 2 * hp + h2
                    nc.tensor.matmul(
                        ot_ps[h2][:],
                        lhsT=V[tb][:, kt % 4, h * DP : (h + 1) * DP],
                        rhs=p_t[:, h2, :],
                        start=(kt == 0),
                        stop=(kt == nkt - 1),
                    )

            # software pipeline: S(kt+1) emitted before AV(kt) so the PE
            # always has matmul work while ACT runs exp(kt+1)
            emit_scores_exp(0)
            for kt in range(1, nkt):
                emit_scores_exp(kt)
                emit_av(kt - 1)
            emit_av(nkt - 1)

            # normalize: divide rows 0..63 by the sums row (64)
            for h2 in range(2):
                recip = recip_pool.tile([1, 512], F32, tag="recip", name="recip")
                nc.vector.reciprocal(recip[:], ot_ps[h2][D : D + 1, :])
                bc = bc_pool.tile([D, 512], F32, tag="bc", name="bc")
                nc.gpsimd.partition_broadcast(bc[:], recip[:])
                nc.vector.tensor_mul(
                    OT[hp][qb][h2 * D : (h2 + 1) * D, :],
                    ot_ps[h2][:D, :],
                    bc[:],
                )

        def proj_tile(tt):
            qb, sub = tt // 4, tt % 4
            y_ps = [ps_y_pool.tile([P, 512], F32, tag="y_ps", name=f"y_ps_{i}") for i in range(2)]
            for ct in range(4):
                lhsT = OT[ct][qb][:, sub * P : (sub + 1) * P]
                for nb in range(2):
                    nc.tensor.matmul(
                        y_ps[nb][:],
                        lhsT=lhsT,
                        rhs=wp_sb[:, ct, nb * 512 : (nb + 1) * 512],
                        start=(ct == 0),
                        stop=(ct == 3),
                    )
            if tt % 2 == 0:
                ypair = y_pool.tile([P, 2, C], F32, tag="ypair", name="ypair")
                ysbs.append(ypair)
            for nb in range(2):
                nc.vector.tensor_copy(
                    ysbs[-1][:, tt % 2, nb * 512 : (nb + 1) * 512], y_ps[nb][:]
                )
            if tt % 2 == 1:  # one DMA per two t-tiles
                nc.sync.dma_start(y_t[:, tt - 1 : tt + 1, :], ysbs[-1][:])

        for qb in range(QB):
            for hp in range(4):
                attention_pair(qb, hp)
            # out-proj for this q-block: fills PE while ACT works on the
            # next q-block's exps
            for tt in range(4 * qb, 4 * qb + 4):
                proj_tile(tt)

    return nc


_CACHED_NC = None


def get_nc():
    global _CACHED_NC
    if _CACHED_NC is None:
        nc = bacc.Bacc()
        with tile.TileContext(nc) as tc:
            build_attention_kernel(tc)
        nc.compile()
        _CACHED_NC = nc
    return _CACHED_NC


def make_in_maps(x, W_att, W_proj):
    x = np.asarray(x, dtype=np.float32)
    W_att = np.asarray(W_att, dtype=np.float32)
    in_maps = []
    for c in range(NC_CORES):
        b, hg = c // 2, c % 2
        s = hg * 512
        in_maps.append(
            {
                "xT": np.ascontiguousarray(x[b].T).astype(BF16_NP),
                "wk": np.ascontiguousarray(
                    W_att[:, 0 * C + s : 0 * C + s + 512]
                ).astype(BF16_NP),
                "wq": np.ascontiguousarray(
                    W_att[:, 1 * C + s : 1 * C + s + 512]
                ).astype(BF16_NP),
                "wv": np.ascontiguousarray(
                    W_att[:, 2 * C + s : 2 * C + s + 512]
                ).astype(BF16_NP),
                "wp": np.ascontiguousarray(
                    np.asarray(W_proj, np.float32)[s : s + 512]
                ).astype(BF16_NP),
            }
        )
    return in_maps


def combine_outputs(results, b_proj):
    B = NC_CORES // 2
    out = np.empty((B, T, C), dtype=np.float32)
    bias = np.asarray(b_proj, dtype=np.float32)
    for b in range(B):
        out[b] = results[2 * b]["y"] + results[2 * b + 1]["y"] + bias
    return out


def kernel(x, W_att, W_proj, b_proj):
    from concourse.bass_utils import run_bass_kernel_spmd

    nc = get_nc()
    in_maps = make_in_maps(x, W_att, W_proj)
    res = run_bass_kernel_spmd(nc, in_maps, list(range(NC_CORES)))
    return combine_outputs(res.results, b_proj)


# revision 12
# speedup vs baseline: 1.2213x; 1.2096x over previous
"""Multi-head causal attention (B=4, T=2048, H=16, D=64) on 8 trn2 NeuronCores.

Sharding: core c = (batch b = c//2, head-group hg = c%2 of 8 heads).
Each core computes its batch's QKV projection for its 8 heads, causal
attention, and a partial output projection (contraction over its 512
channels of W_proj). Host sums the two partials per batch and adds bias.

Per-core kernel layout choices (see comments inline):
  - x is passed pre-transposed as xT [C=1024, T=2048].
  - K^T, Q^T stored [hd, t] with head-dim on partitions (64 per head, 2
    heads per 128-partition tile) -> scores matmuls row-pack 2 heads.
  - V stored [t, h*65+d] with a ones column appended per head -> the AV
    matmul O_T = V_aug^T(stationary) x P_T produces softmax denominators
    in row 64 for free.
  - Scores are computed transposed S_T[k, q] so that P_T = exp(S_T) is
    directly the AV matmul's moving operand; softmax needs no max
    subtraction (|scores/8| < ~3) and no separate sum reduction.
  - O_T [hd, t] is exactly the lhsT the output projection needs.
"""

import os
import sys

import numpy as np

BF16_NP = np.dtype(np.float16)

if "/opt/trn_rl_repo" not in sys.path:
    sys.path.insert(0, "/opt/trn_rl_repo")

from contextlib import ExitStack

import concourse.bass as bass
import concourse.bacc as bacc
import concourse.mybir as mybir
import concourse.tile as tile
from concourse._compat import with_exitstack

P = 128
T = 2048
C = 1024
H_PER_CORE = 8
D = 64
DP = D + 1  # V augmented with a ones column
NC_CORES = 8

TB = 4  # t-blocks of 512
QB = 4  # q-blocks of 512
NKT = 16  # k-tiles of 128
CI = 8  # contraction tiles of 128 over C for QKV proj

F32 = mybir.dt.float32
BF16 = mybir.dt.float16  # fp16: full matmul rate, 8x finer mantissa than bf16


def _mm(ap):
    return ap


@with_exitstack
def build_attention_kernel(ctx: ExitStack, tc: tile.TileContext):
    nc = tc.nc

    xT = nc.declare_dram_parameter("xT", [C, T], BF16, isOutput=False)
    wk = nc.declare_dram_parameter("wk", [C, 512], BF16, isOutput=False)
    wq = nc.declare_dram_parameter("wq", [C, 512], BF16, isOutput=False)
    wv = nc.declare_dram_parameter("wv", [C, 512], BF16, isOutput=False)
    wp = nc.declare_dram_parameter("wp", [512, C], BF16, isOutput=False)
    y = nc.declare_dram_parameter("y", [T, C], F32, isOutput=True)

    xT_t = xT.rearrange("(co ci) t -> ci co t", ci=P)
    wk_t = wk.rearrange("(co ci) m -> ci co m", ci=P)
    wq_t = wq.rearrange("(co ci) m -> ci co m", ci=P)
    wv_t = wv.rearrange("(co ci) m -> ci co m", ci=P)
    wp_t = wp.rearrange("(co ci) n -> ci co n", ci=P)

    # ---- persistent SBUF pools ----
    # Fine-grained tiles so phase-2 consumers only wait on the exact
    # phase-1 producers they need.
    kt_pool = ctx.enter_context(tc.tile_pool(name="ktp", bufs=16))
    qt_pool = ctx.enter_context(tc.tile_pool(name="qtp", bufs=16))
    v_pool = ctx.enter_context(tc.tile_pool(name="vp", bufs=4))
    const_pool = ctx.enter_context(tc.tile_pool(name="constp", bufs=1))

    # KT[pt][tb], QT[pt][qb]: [128, 512]; partitions = 2 heads x 64 dims
    KT = [[kt_pool.tile([P, 512], BF16, tag="kt", name=f"KT_{pt}_{tb}") for tb in range(TB)] for pt in range(4)]
    QT = [[qt_pool.tile([P, 512], BF16, tag="qt", name=f"QT_{pt}_{qb}") for qb in range(QB)] for pt in range(4)]
    # V[tb]: [128, 4(t-subtile), 8*65]
    V = [v_pool.tile([P, 4, H_PER_CORE * DP], BF16, tag="v", name=f"V_{tb}") for tb in range(TB)]

    masks = const_pool.tile([P, 4, 512], BF16, tag="masks", name="masks")

    # diagonal causal masks: masks[:, j, :][kk, qq] = 1.0 if qq >= kk + j*128
    for j in range(4):
        nc.gpsimd.memset(masks[:, j, :], 1.0)
        nc.gpsimd.affine_select(
            out=masks[:, j, :],
            in_=masks[:, j, :],
            compare_op=mybir.AluOpType.is_ge,
            fill=0.0,
            base=-j * P,
            pattern=[[1, 512]],  # +1 per q element
            channel_multiplier=-1,  # -1 per k partition
        )
    # ones column of V
    for tb in range(TB):
        ones_col = V[tb].rearrange("p s (h e) -> p s h e", e=DP)[:, :, :, D : D + 1]
        nc.gpsimd.memset(ones_col, 1.0)

    # =========================== phase 1: QKV ===========================
    with (
        tc.tile_pool(name="wkp", bufs=1) as wk_pool,
        tc.tile_pool(name="wqp", bufs=1) as wq_pool,
        tc.tile_pool(name="wvp", bufs=1) as wv_pool,
        tc.tile_pool(name="xtp", bufs=2) as xt_pool,
        tc.tile_pool(name="ps1", bufs=4, space="PSUM") as ps1_pool,
    ):
        wk_sb = wk_pool.tile([P, CI, 512], BF16)
        wq_sb = wq_pool.tile([P, CI, 512], BF16)
        wv_sb = wv_pool.tile([P, CI, 512], BF16)
        nc.sync.dma_start(wk_sb[:], wk_t)
        xts = []
        for tb in range(2):  # prefetch first two t-blocks before wq/wv
            xt = xt_pool.tile([P, CI, 512], BF16, tag="xt", name="xt")
            nc.sync.dma_start(xt[:], xT_t[:, :, tb * 512 : (tb + 1) * 512])
            xts.append(xt)
        nc.sync.dma_start(wq_sb[:], wq_t)
        nc.sync.dma_start(wv_sb[:], wv_t)

        for tb in range(TB):
            if tb < 2:
                xt = xts[tb]
            else:
                xt = xt_pool.tile([P, CI, 512], BF16, tag="xt", name="xt")
                nc.sync.dma_start(xt[:], xT_t[:, :, tb * 512 : (tb + 1) * 512])

            for pt in range(4):  # K^T: lhsT = wk cols, rhs = xT
                ps = ps1_pool.tile([P, 512], F32, tag="ps1", name="ps")
                for ci in range(CI):
                    nc.tensor.matmul(
                        ps[:],
                        lhsT=_mm(wk_sb[:, ci, pt * P : (pt + 1) * P]),
                        rhs=_mm(xt[:, ci, :]),
                        start=(ci == 0),
                        stop=(ci == CI - 1),
                    )
                nc.scalar.copy(KT[pt][tb][:], ps[:])
            for pt in range(4):  # Q^T
                ps = ps1_pool.tile([P, 512], F32, tag="ps1", name="ps")
                for ci in range(CI):
                    nc.tensor.matmul(
                        ps[:],
                        lhsT=_mm(wq_sb[:, ci, pt * P : (pt + 1) * P]),
                        rhs=_mm(xt[:, ci, :]),
                        start=(ci == 0),
                        stop=(ci == CI - 1),
                    )
                nc.vector.tensor_copy(QT[pt][tb][:], ps[:])
            for ts_ in range(4):  # V: lhsT = xT t-slice, rhs = wv
                ps = ps1_pool.tile([P, 512], F32, tag="ps1", name="ps")
                for ci in range(CI):
                    nc.tensor.matmul(
                        ps[:],
                        lhsT=_mm(xt[:, ci, ts_ * P : (ts_ + 1) * P]),
                        rhs=_mm(wv_sb[:, ci, :]),
                        start=(ci == 0),
                        stop=(ci == CI - 1),
                    )
                nc.vector.tensor_copy(
                    V[tb][:, ts_].rearrange("p (h e) -> p h e", e=DP)[:, :, :D],
                    ps.rearrange("p (h d) -> p h d", d=D),
                )

    # ================== phase 2+3: attention + out-proj ==================
    # OT pool opened after phase-1 pools closed: reuses their SBUF space.
    ot_pool = ctx.enter_context(tc.tile_pool(name="otp", bufs=16))
    # OT[hp][qb]: [128, 512] attention out (normalized), same layout as KT
    OT = [[ot_pool.tile([P, 512], BF16, tag="ot", name=f"OT_{hp}_{qb}") for qb in range(QB)] for hp in range(4)]
    with (
        tc.tile_pool(name="ptp", bufs=4) as pt_pool,
        tc.tile_pool(name="recipp", bufs=4) as recip_pool,
        tc.tile_pool(name="bcp", bufs=2) as bc_pool,
        tc.tile_pool(name="wpp", bufs=1) as wp_pool,
        tc.tile_pool(name="yp", bufs=2) as y_pool,
        tc.tile_pool(name="ps_s", bufs=2, space="PSUM") as ps_s_pool,
        tc.tile_pool(name="ps_o", bufs=2, space="PSUM") as ps_o_pool,
        tc.tile_pool(name="ps_y", bufs=2, space="PSUM") as ps_y_pool,
    ):
        wp_sb = wp_pool.tile([P, 4, C], BF16)
        nc.sync.dma_start(wp_sb[:], wp_t)
        y_t = y.rearrange("(tt p) n -> p tt n", p=P)
        ysbs = []

        def attention_pair(qb, hp):
            ot_ps = [ps_o_pool.tile([DP, 512], F32, tag="ot_ps", name=f"ot_ps_{i}") for i in range(2)]
            nkt = 4 * (qb + 1)
            pts = {}

            def emit_scores_exp(kt):
                tb = kt // 4
                # both heads' S_T side by side in one 2-bank psum tile
                s_ps = ps_s_pool.tile([P, 2, 512], F32, tag="s_ps", name="s_ps")
                for h2 in range(2):
                    # S_T[k, q] for head h = 2*hp + h2 (row-packed pair)
                    nc.tensor.matmul(
                        s_ps[:, h2, :],
                        lhsT=KT[hp][tb][
                            h2 * D : (h2 + 1) * D,
                            (kt % 4) * P : (kt % 4 + 1) * P,
                        ],
                        rhs=QT[hp][qb][h2 * D : (h2 + 1) * D, :],
                        start=True,
                        stop=True,
                    )
                # one exp over both heads (2 banks, 1024 elems/partition)
                p_t = pt_pool.tile([P, 2, 512], BF16, tag="pt", name="p_t")
                nc.scalar.activation(
                    p_t[:],
                    s_ps[:],
                    mybir.ActivationFunctionType.Exp,
                    scale=0.125,
                )
                if kt >= 4 * qb:  # diagonal tile: zero q < k entries
                    j = kt - 4 * qb
                    mb = masks[:, j : j + 1, :].to_broadcast([P, 2, 512])
                    nc.vector.tensor_mul(p_t[:], p_t[:], mb)
                pts[kt] = p_t

            def emit_av(kt):
                tb = kt // 4
                p_t = pts.pop(kt)
                for h2 in range(2):
                    h = 2 * hp + h2
                    nc.tensor.matmul(
                        ot_ps[h2][:],
                        lhsT=V[tb][:, kt % 4, h * DP : (h + 1) * DP],
                        rhs=p_t[:, h2, :],
                        start=(kt == 0),
                        stop=(kt == nkt - 1),
                    )

            # software pipeline: S(kt+1) emitted before AV(kt) so the PE
            # always has matmul work while ACT runs exp(kt+1)
            emit_scores_exp(0)
            for kt in range(1, nkt):
                emit_scores_exp(kt)
                emit_av(kt - 1)
            emit_av(nkt - 1)

            # normalize: divide rows 0..63 by the sums row (64)
            for h2 in range(2):
                recip = recip_pool.tile([1, 512], F32, tag="recip", name="recip")
                nc.vector.tensor_copy(recip[:], ot_ps[h2][D : D + 1, :])
                nc.vector.reciprocal_approx_fast(recip[:], recip[:])
                bc = bc_pool.tile([D, 512], F32, tag="bc", name="bc")
                nc.gpsimd.partition_broadcast(bc[:], recip[:])
                nc.vector.tensor_mul(
                    OT[hp][qb][h2 * D : (h2 + 1) * D, :],
                    ot_ps[h2][:D, :],
                    bc[:],
                )

        def proj_tile(tt):
            qb, sub = tt // 4, tt % 4
            y_ps = [ps_y_pool.tile([P, 512], F32, tag="y_ps", name=f"y_ps_{i}") for i in range(2)]
            for ct in range(4):
                lhsT = OT[ct][qb][:, sub * P : (sub + 1) * P]
                for nb in range(2):
                    nc.tensor.matmul(
                        y_ps[nb][:],
                        lhsT=lhsT,
                        rhs=wp_sb[:, ct, nb * 512 : (nb + 1) * 512],
                        start=(ct == 0),
                        stop=(ct == 3),
                    )
            if tt % 2 == 0:
                ypair = y_pool.tile([P, 2, C], F32, tag="ypair", name="ypair")
                ysbs.append(ypair)
            for nb in range(2):
                nc.vector.tensor_copy(
                    ysbs[-1][:, tt % 2, nb * 512 : (nb + 1) * 512], y_ps[nb][:]
                )
            if tt % 2 == 1:  # one DMA per two t-tiles
                nc.sync.dma_start(y_t[:, tt - 1 : tt + 1, :], ysbs[-1][:])

        for qb in range(QB):
            for hp in range(4):
                attention_pair(qb, hp)
            # out-proj for this q-block: fills PE while ACT works on the
            # next q-block's exps
            for tt in range(4 * qb, 4 * qb + 4):
                proj_tile(tt)

    return nc


_CACHED_NC = None


def get_nc():
    global _CACHED_NC
    if _CACHED_NC is None:
        nc = bacc.Bacc()
        with tile.TileContext(nc) as tc:
            build_attention_kernel(tc)
        nc.compile()
        _CACHED_NC = nc
    return _CACHED_NC


def make_in_maps(x, W_att, W_proj):
    x = np.asarray(x, dtype=np.float32)
    W_att = np.asarray(W_att, dtype=np.float32)
    in_maps = []
    for c in range(NC_CORES):
        b, hg = c // 2, c % 2
        s = hg * 512
        in_maps.append(
            {
                "xT": np.ascontiguousarray(x[b].T).astype(BF16_NP),
                "wk": np.ascontiguousarray(
                    W_att[:, 0 * C + s : 0 * C + s + 512]
                ).astype(BF16_NP),
                "wq": np.ascontiguousarray(
                    W_att[:, 1 * C + s : 1 * C + s + 512]
                ).astype(BF16_NP),
                "wv": np.ascontiguousarray(
                    W_att[:, 2 * C + s : 2 * C + s + 512]
                ).astype(BF16_NP),
                "wp": np.ascontiguousarray(
                    np.asarray(W_proj, np.float32)[s : s + 512]
                ).astype(BF16_NP),
            }
        )
    return in_maps


def combine_outputs(results, b_proj):
    B = NC_CORES // 2
    out = np.empty((B, T, C), dtype=np.float32)
    bias = np.asarray(b_proj, dtype=np.float32)
    for b in range(B):
        out[b] = results[2 * b]["y"] + results[2 * b + 1]["y"] + bias
    return out


def kernel(x, W_att, W_proj, b_proj):
    from concourse.bass_utils import run_bass_kernel_spmd

    nc = get_nc()
    in_maps = make_in_maps(x, W_att, W_proj)
    res = run_bass_kernel_spmd(nc, in_maps, list(range(NC_CORES)))
    return combine_outputs(res.results, b_proj)


# revision 13
# speedup vs baseline: 1.3351x; 1.0932x over previous
"""Multi-head causal attention (B=4, T=2048, H=16, D=64) on 8 trn2 NeuronCores.

Sharding: core c = (batch b = c//2, head-group hg = c%2 of 8 heads).
Each core computes its batch's QKV projection for its 8 heads, causal
attention, and a partial output projection (contraction over its 512
channels of W_proj). Host sums the two partials per batch and adds bias.

Per-core kernel layout choices (see comments inline):
  - x is passed pre-transposed as xT [C=1024, T=2048].
  - K^T, Q^T stored [hd, t] with head-dim on partitions (64 per head, 2
    heads per 128-partition tile) -> scores matmuls row-pack 2 heads.
  - V stored [t, h*65+d] with a ones column appended per head -> the AV
    matmul O_T = V_aug^T(stationary) x P_T produces softmax denominators
    in row 64 for free.
  - Scores are computed transposed S_T[k, q] so that P_T = exp(S_T) is
    directly the AV matmul's moving operand; softmax needs no max
    subtraction (|scores/8| < ~3) and no separate sum reduction.
  - O_T [hd, t] is exactly the lhsT the output projection needs.
"""

import os
import sys

import numpy as np

BF16_NP = np.dtype(np.float16)

if "/opt/trn_rl_repo" not in sys.path:
    sys.path.insert(0, "/opt/trn_rl_repo")

from contextlib import ExitStack

import concourse.bass as bass
import concourse.bacc as bacc
import concourse.mybir as mybir
import concourse.tile as tile
from concourse._compat import with_exitstack

P = 128
T = 2048
C = 1024
H_PER_CORE = 8
D = 64
DP = D + 1  # V augmented with a ones column
NC_CORES = 8

TB = 4  # t-blocks of 512
QB = 4  # q-blocks of 512
NKT = 16  # k-tiles of 128
CI = 8  # contraction tiles of 128 over C for QKV proj

F32 = mybir.dt.float32
BF16 = mybir.dt.float16  # fp16: full matmul rate, 8x finer mantissa than bf16


def _mm(ap):
    return ap


@with_exitstack
def build_attention_kernel(ctx: ExitStack, tc: tile.TileContext):
    nc = tc.nc

    xT = nc.declare_dram_parameter("xT", [C, T], BF16, isOutput=False)
    wk = nc.declare_dram_parameter("wk", [C, 512], BF16, isOutput=False)
    wq = nc.declare_dram_parameter("wq", [C, 512], BF16, isOutput=False)
    wv = nc.declare_dram_parameter("wv", [C, 512], BF16, isOutput=False)
    wp = nc.declare_dram_parameter("wp", [512, C], BF16, isOutput=False)
    y = nc.declare_dram_parameter("y", [T, C], F32, isOutput=True)

    xT_t = xT.rearrange("(co ci) t -> ci co t", ci=P)
    wk_t = wk.rearrange("(co ci) m -> ci co m", ci=P)
    wq_t = wq.rearrange("(co ci) m -> ci co m", ci=P)
    wv_t = wv.rearrange("(co ci) m -> ci co m", ci=P)
    wp_t = wp.rearrange("(co ci) n -> ci co n", ci=P)

    # ---- persistent SBUF pools ----
    # Fine-grained tiles so phase-2 consumers only wait on the exact
    # phase-1 producers they need.
    kt_pool = ctx.enter_context(tc.tile_pool(name="ktp", bufs=16))
    qt_pool = ctx.enter_context(tc.tile_pool(name="qtp", bufs=16))
    v_pool = ctx.enter_context(tc.tile_pool(name="vp", bufs=4))
    const_pool = ctx.enter_context(tc.tile_pool(name="constp", bufs=1))

    # KT[pt][tb], QT[pt][qb]: [128, 512]; partitions = 2 heads x 64 dims
    KT = [[kt_pool.tile([P, 512], BF16, tag="kt", name=f"KT_{pt}_{tb}") for tb in range(TB)] for pt in range(4)]
    QT = [[qt_pool.tile([P, 512], BF16, tag="qt", name=f"QT_{pt}_{qb}") for qb in range(QB)] for pt in range(4)]
    # V[tb]: [128, 4(t-subtile), 8*65]
    V = [v_pool.tile([P, 4, H_PER_CORE * DP], BF16, tag="v", name=f"V_{tb}") for tb in range(TB)]

    masks = const_pool.tile([P, 4, 512], BF16, tag="masks", name="masks")

    # diagonal causal masks: masks[:, j, :][kk, qq] = 1.0 if qq >= kk + j*128
    for j in range(4):
        nc.gpsimd.memset(masks[:, j, :], 1.0)
        nc.gpsimd.affine_select(
            out=masks[:, j, :],
            in_=masks[:, j, :],
            compare_op=mybir.AluOpType.is_ge,
            fill=0.0,
            base=-j * P,
            pattern=[[1, 512]],  # +1 per q element
            channel_multiplier=-1,  # -1 per k partition
        )
    # ones column of V
    for tb in range(TB):
        ones_col = V[tb].rearrange("p s (h e) -> p s h e", e=DP)[:, :, :, D : D + 1]
        nc.gpsimd.memset(ones_col, 1.0)

    # =========================== phase 1: QKV ===========================
    with (
        tc.tile_pool(name="wkp", bufs=1) as wk_pool,
        tc.tile_pool(name="wqp", bufs=1) as wq_pool,
        tc.tile_pool(name="wvp", bufs=1) as wv_pool,
        tc.tile_pool(name="xtp", bufs=2) as xt_pool,
        tc.tile_pool(name="ps1", bufs=4, space="PSUM") as ps1_pool,
    ):
        wk_sb = wk_pool.tile([P, CI, 512], BF16)
        wq_sb = wq_pool.tile([P, CI, 512], BF16)
        wv_sb = wv_pool.tile([P, CI, 512], BF16)
        nc.sync.dma_start(wk_sb[:, :4], wk_t[:, :4])
        xts = []
        xt0 = xt_pool.tile([P, CI, 512], BF16, tag="xt", name="xt")
        nc.sync.dma_start(xt0[:, :4], xT_t[:, :4, 0:512])
        nc.sync.dma_start(wk_sb[:, 4:], wk_t[:, 4:])
        nc.sync.dma_start(xt0[:, 4:], xT_t[:, 4:, 0:512])
        xts.append(xt0)
        xt1 = xt_pool.tile([P, CI, 512], BF16, tag="xt", name="xt")
        nc.sync.dma_start(xt1[:], xT_t[:, :, 512:1024])
        xts.append(xt1)
        nc.sync.dma_start(wq_sb[:], wq_t)
        nc.sync.dma_start(wv_sb[:], wv_t)

        for tb in range(TB):
            if tb < 2:
                xt = xts[tb]
            else:
                xt = xt_pool.tile([P, CI, 512], BF16, tag="xt", name="xt")
                nc.sync.dma_start(xt[:], xT_t[:, :, tb * 512 : (tb + 1) * 512])

            for pt in range(4):  # K^T: lhsT = wk cols, rhs = xT
                ps = ps1_pool.tile([P, 512], F32, tag="ps1", name="ps")
                for ci in range(CI):
                    nc.tensor.matmul(
                        ps[:],
                        lhsT=_mm(wk_sb[:, ci, pt * P : (pt + 1) * P]),
                        rhs=_mm(xt[:, ci, :]),
                        start=(ci == 0),
                        stop=(ci == CI - 1),
                    )
                nc.scalar.copy(KT[pt][tb][:], ps[:])
            for pt in range(4):  # Q^T
                ps = ps1_pool.tile([P, 512], F32, tag="ps1", name="ps")
                for ci in range(CI):
                    nc.tensor.matmul(
                        ps[:],
                        lhsT=_mm(wq_sb[:, ci, pt * P : (pt + 1) * P]),
                        rhs=_mm(xt[:, ci, :]),
                        start=(ci == 0),
                        stop=(ci == CI - 1),
                    )
                nc.vector.tensor_copy(QT[pt][tb][:], ps[:])
            for ts_ in range(4):  # V: lhsT = xT t-slice, rhs = wv
                ps = ps1_pool.tile([P, 512], F32, tag="ps1", name="ps")
                for ci in range(CI):
                    nc.tensor.matmul(
                        ps[:],
                        lhsT=_mm(xt[:, ci, ts_ * P : (ts_ + 1) * P]),
                        rhs=_mm(wv_sb[:, ci, :]),
                        start=(ci == 0),
                        stop=(ci == CI - 1),
                    )
                nc.vector.tensor_copy(
                    V[tb][:, ts_].rearrange("p (h e) -> p h e", e=DP)[:, :, :D],
                    ps.rearrange("p (h d) -> p h d", d=D),
                )

    # ================== phase 2+3: attention + out-proj ==================
    # OT pool opened after phase-1 pools closed: reuses their SBUF space.
    ot_pool = ctx.enter_context(tc.tile_pool(name="otp", bufs=16))
    # OT[hp][qb]: [128, 512] attention out (normalized), same layout as KT
    OT = [[ot_pool.tile([P, 512], BF16, tag="ot", name=f"OT_{hp}_{qb}") for qb in range(QB)] for hp in range(4)]
    with (
        tc.tile_pool(name="ptp", bufs=4) as pt_pool,
        tc.tile_pool(name="recipp", bufs=4) as recip_pool,
        tc.tile_pool(name="bcp", bufs=2) as bc_pool,
        tc.tile_pool(name="wpp", bufs=1) as wp_pool,
        tc.tile_pool(name="yp", bufs=2) as y_pool,
        tc.tile_pool(name="ps_s", bufs=3, space="PSUM") as ps_s_pool,
        tc.tile_pool(name="ps_o", bufs=2, space="PSUM") as ps_o_pool,
    ):
        wp_sb = wp_pool.tile([P, 4, C], BF16)
        nc.sync.dma_start(wp_sb[:], wp_t)
        y_t = y.rearrange("(tt p) n -> p tt n", p=P)
        ysbs = []

        def attention_pair(qb, hp):
            ot_ps = [ps_o_pool.tile([DP, 512], F32, tag="ot_ps", name=f"ot_ps_{i}") for i in range(2)]
            nkt = 4 * (qb + 1)
            pts = {}

            def emit_scores_exp(kt):
                tb = kt // 4
                # diagonal tiles: only q >= j*128 is (partially) visible;
                # restrict all work to that slice
                qs = (kt - 4 * qb) * P if kt >= 4 * qb else 0
                nq = 512 - qs
                # both heads' S_T side by side in one 2-bank psum tile
                s_ps = ps_s_pool.tile([P, 2, 512], F32, tag="s_ps", name="s_ps")
                for h2 in range(2):
                    # S_T[k, q] for head h = 2*hp + h2 (row-packed pair)
                    nc.tensor.matmul(
                        s_ps[:, h2, qs:],
                        lhsT=KT[hp][tb][
                            h2 * D : (h2 + 1) * D,
                            (kt % 4) * P : (kt % 4 + 1) * P,
                        ],
                        rhs=QT[hp][qb][h2 * D : (h2 + 1) * D, qs:],
                        start=True,
                        stop=True,
                    )
                # one exp over both heads (2 banks)
                p_t = pt_pool.tile([P, 2, 512], BF16, tag="pt", name="p_t")
                nc.scalar.activation(
                    p_t[:, :, qs:],
                    s_ps[:, :, qs:],
                    mybir.ActivationFunctionType.Exp,
                    scale=0.125,
                )
                if kt >= 4 * qb:  # diagonal tile: zero q < k entries
                    j = kt - 4 * qb
                    mb = masks[:, j : j + 1, qs:].to_broadcast([P, 2, nq])
                    nc.vector.tensor_mul(p_t[:, :, qs:], p_t[:, :, qs:], mb)
                pts[kt] = (p_t, qs)

            def emit_av(kt):
                tb = kt // 4
                p_t, qs = pts.pop(kt)
                for h2 in range(2):
                    h = 2 * hp + h2
                    nc.tensor.matmul(
                        ot_ps[h2][:, qs:],
                        lhsT=V[tb][:, kt % 4, h * DP : (h + 1) * DP],
                        rhs=p_t[:, h2, qs:],
                        start=(kt == 0),
                        stop=(kt == nkt - 1),
                    )

            # software pipeline: S(kt+1) emitted before AV(kt) so the PE
            # always has matmul work while ACT runs exp(kt+1)
            emit_scores_exp(0)
            for kt in range(1, nkt):
                emit_scores_exp(kt)
                emit_av(kt - 1)
            emit_av(nkt - 1)

            # normalize: divide rows 0..63 by the sums row (64)
            for h2 in range(2):
                recip = recip_pool.tile([1, 512], F32, tag="recip", name="recip")
                nc.vector.tensor_copy(recip[:], ot_ps[h2][D : D + 1, :])
                nc.vector.reciprocal_approx_fast(recip[:], recip[:])
                bc = bc_pool.tile([D, 512], F32, tag="bc", name="bc")
                nc.gpsimd.partition_broadcast(bc[:], recip[:])
                nc.vector.tensor_mul(
                    OT[hp][qb][h2 * D : (h2 + 1) * D, :],
                    ot_ps[h2][:D, :],
                    bc[:],
                )

        def proj_tile(tt):
            qb, sub = tt // 4, tt % 4
            y_pair_ps = ps_s_pool.tile([P, 2, 512], F32, tag="s_ps", name="y_ps")
            y_ps = [y_pair_ps[:, 0, :], y_pair_ps[:, 1, :]]
            for ct in range(4):
                lhsT = OT[ct][qb][:, sub * P : (sub + 1) * P]
                for nb in range(2):
                    nc.tensor.matmul(
                        y_ps[nb],
                        lhsT=lhsT,
                        rhs=wp_sb[:, ct, nb * 512 : (nb + 1) * 512],
                        start=(ct == 0),
                        stop=(ct == 3),
                    )
            if tt % 2 == 0:
                ypair = y_pool.tile([P, 2, C], F32, tag="ypair", name="ypair")
                ysbs.append(ypair)
            for nb in range(2):
                nc.vector.tensor_copy(
                    ysbs[-1][:, tt % 2, nb * 512 : (nb + 1) * 512], y_ps[nb]
                )
            if tt % 2 == 1:  # one DMA per two t-tiles
                nc.sync.dma_start(y_t[:, tt - 1 : tt + 1, :], ysbs[-1][:])

        for qb in range(QB):
            for hp in range(4):
                attention_pair(qb, hp)
            # out-proj for this q-block: fills PE while ACT works on the
            # next q-block's exps
            for tt in range(4 * qb, 4 * qb + 4):
                proj_tile(tt)

    return nc


_CACHED_NC = None


def get_nc():
    global _CACHED_NC
    if _CACHED_NC is None:
        nc = bacc.Bacc()
        with tile.TileContext(nc) as tc:
            build_attention_kernel(tc)
        nc.compile()
        _CACHED_NC = nc
    return _CACHED_NC


def make_in_maps(x, W_att, W_proj):
    x = np.asarray(x, dtype=np.float32)
    W_att = np.asarray(W_att, dtype=np.float32)
    in_maps = []
    for c in range(NC_CORES):
        b, hg = c // 2, c % 2
        s = hg * 512
        in_maps.append(
            {
                "xT": np.ascontiguousarray(x[b].T).astype(BF16_NP),
                "wk": np.ascontiguousarray(
                    W_att[:, 0 * C + s : 0 * C + s + 512]
                ).astype(BF16_NP),
                "wq": np.ascontiguousarray(
                    W_att[:, 1 * C + s : 1 * C + s + 512]
                ).astype(BF16_NP),
                "wv": np.ascontiguousarray(
                    W_att[:, 2 * C + s : 2 * C + s + 512]
                ).astype(BF16_NP),
                "wp": np.ascontiguousarray(
                    np.asarray(W_proj, np.float32)[s : s + 512]
                ).astype(BF16_NP),
            }
        )
    return in_maps


def combine_outputs(results, b_proj):
    B = NC_CORES // 2
    out = np.empty((B, T, C), dtype=np.float32)
    bias = np.asarray(b_proj, dtype=np.float32)
    for b in range(B):
        out[b] = results[2 * b]["y"] + results[2 * b + 1]["y"] + bias
    return out


def kernel(x, W_att, W_proj, b_proj):
    from concourse.bass_utils import run_bass_kernel_spmd

    nc = get_nc()
    in_maps = make_in_maps(x, W_att, W_proj)
    res = run_bass_kernel_spmd(nc, in_maps, list(range(NC_CORES)))
    return combine_outputs(res.results, b_proj)
